# revision 15
# baseline (speedup 1.0000x reference)
"""GAT block (GATConv + InstanceNorm + residual + ELU) on 8 Trainium2 NeuronCores.

Strategy (graph/data parallel, per the dst-node partition scheme):
  - Host routes each edge to the core owning its dst node (node range split).
  - Per core, dst nodes are degree-sorted and grouped into tiles of 128;
    each node's incoming edges live in padded "slots" (slot 0 = self loop,
    pads point at a dummy table row whose a_src = -1e30 so softmax kills it).
  - Each core builds the full gather table hx = x @ [W | w_src | w_dst]
    ([N+1, F+2H]) in its own DRAM, then per tile indirect-DMA-gathers the
    src rows of all slots, computes per-slot attention logits, softmax over
    slots, and the alpha-weighted feature sum -- dst node == partition, so
    no scatter is ever needed.
  - a_edge = edge_attr @ v (v = W_e . att_edge folded on host) is computed
    with TensorE on a host-transposed, 4-slot-interleaved layout; the self
    loop's a_edge is (sum_k a_edge_k) / deg  (linearity of a_edge in ea).
  - InstanceNorm stats via ones-matmul partition reduction, AllReduce'd
    across the 8 cores; finalization (affine + residual + ELU) per tile.
"""

import math
import numpy as np

# ---------------------------------------------------------------- constants
P = 128  # partitions


def _cfg_full():
    return dict(N=50000, E=1600000, F=128, H=8, Dh=16, ED=16, NC=8)


def _fold_weights(W, att_src, att_dst, W_e, att_edge, H, Dh):
    F = W.shape[0]
    w_src = np.stack(
        [W[:, h * Dh:(h + 1) * Dh] @ att_src[h] for h in range(H)], axis=1
    )  # [F, H]
    w_dst = np.stack(
        [W[:, h * Dh:(h + 1) * Dh] @ att_dst[h] for h in range(H)], axis=1
    )
    Wb = np.concatenate([W, w_src, w_dst], axis=1).astype(np.float32)  # [F, F+2H]
    v = np.stack(
        [W_e[:, h * Dh:(h + 1) * Dh] @ att_edge[h] for h in range(H)], axis=1
    ).astype(np.float32)  # [ED, H]
    ED = W_e.shape[0]
    v4 = np.zeros((4 * ED, 4 * H), dtype=np.float32)
    for j in range(4):
        v4[j * ED:(j + 1) * ED, j * H:(j + 1) * H] = v
    return Wb, v4


def _preprocess(edge_index, edge_attr, cfg):
    """Route edges to dst owners; build per-core slot arrays with a COMMON
    static tile structure (K per tile) across all cores (SPMD program)."""
    N, E, ED, NC = cfg["N"], cfg["E"], cfg["ED"], cfg["NC"]
    Np = N // NC
    n_tiles = math.ceil(Np / P)
    src = np.asarray(edge_index[0], dtype=np.int64).astype(np.int32)
    dst = np.asarray(edge_index[1], dtype=np.int64).astype(np.int32)
    ea = np.asarray(edge_attr, dtype=np.float32)

    cores = []
    for c in range(NC):
        m = (dst >= c * Np) & (dst < (c + 1) * Np)
        e_ids = np.nonzero(m)[0]
        dst_c = dst[e_ids] - c * Np
        order_e = np.argsort(dst_c, kind="stable")
        e_ids = e_ids[order_e]
        dst_c = dst_c[order_e]
        deg = np.bincount(dst_c, minlength=Np).astype(np.int64)
        cum = np.zeros(Np + 1, dtype=np.int64)
        np.cumsum(deg, out=cum[1:])
        # degree-descending node order, padded to n_tiles*P with -1
        node_order = np.argsort(-deg, kind="stable")
        pad_nodes = n_tiles * P - Np
        node_order_p = np.concatenate(
            [node_order, np.full(pad_nodes, -1, dtype=np.int64)]
        )
        # per-tile max slots (deg+1)
        Ks = []
        for t in range(n_tiles):
            nt = node_order_p[t * P:(t + 1) * P]
            real = nt[nt >= 0]
            kmax = int(deg[real].max()) + 1 if len(real) else 1
            Ks.append(kmax)
        cores.append(dict(e_ids=e_ids, dst_c=dst_c, deg=deg, cum=cum,
                          node_order=node_order_p, Ks=Ks))

    # common K per tile index: max over cores, round up to multiple of 4
    Ks = []
    for t in range(n_tiles):
        k = max(c["Ks"][t] for c in cores)
        Ks.append(((k + 3) // 4) * 4)
    offs = np.zeros(n_tiles + 1, dtype=np.int64)
    np.cumsum(np.array(Ks) * P, out=offs[1:])
    S = int(offs[-1])  # total slots per core
    offs4 = offs // 4  # eaT4 column offsets (4 slots per column-block)
    S4 = S // 4

    for c in range(NC):
        st = cores[c]
        deg, cum, node_order = st["deg"], st["cum"], st["node_order"]
        idx_flat = np.full(S, N, dtype=np.int32)        # dummy row
        eaT4 = np.zeros((4 * ED, S4), dtype=np.float32)
        rdeg = np.ones(n_tiles * P, dtype=np.float32)
        idx0 = np.zeros(n_tiles * P, dtype=np.int32)    # pads -> row 0 of x
        # vectors over tile-position space
        tile_of_pos = np.repeat(np.arange(n_tiles), P)
        p_of_pos = np.tile(np.arange(P), n_tiles)
        real_m = node_order >= 0
        nodes = node_order[real_m]
        g_ids = (c * Np + nodes).astype(np.int32)
        idx0[real_m] = g_ids
        rdeg[real_m] = 1.0 / np.maximum(deg[nodes], 1).astype(np.float32)
        # slot 0 (self loop): index = own global id, at flat pos off+p*K+0
        pos_r = np.nonzero(real_m)[0]
        t_r, p_r = tile_of_pos[pos_r], p_of_pos[pos_r]
        Karr = np.array(Ks, dtype=np.int64)
        idx_flat[offs[t_r] + p_r * Karr[t_r]] = g_ids
        # edges: node n sits at tile/pos (invert node_order)
        pos_of_node = np.empty(Np, dtype=np.int64)
        pos_of_node[nodes] = pos_r
        e_pos = pos_of_node[st["dst_c"]]               # per (sorted) edge
        e_t = tile_of_pos[e_pos]
        e_p = p_of_pos[e_pos]
        k_e = 1 + (np.arange(len(st["dst_c"])) - cum[st["dst_c"]])
        idx_flat[offs[e_t] + e_p * Karr[e_t] + k_e] = src[st["e_ids"]]
        # eaT4: column offs4[t] + (k//4)*P + p ; rows (k%4)*ED + f
        col = offs4[e_t] + (k_e >> 2) * P + e_p
        j = (k_e & 3).astype(np.int64)
        ea_c = ea[st["e_ids"]]
        for jj in range(4):
            mj = j == jj
            eaT4[jj * ED:(jj + 1) * ED, col[mj]] = ea_c[mj].T
        st["in"] = dict(idx=idx_flat, eaT4=eaT4, rdeg=rdeg, idx0=idx0)
    return cores, Ks, offs, offs4


# ---------------------------------------------------------------- device
def _build(cfg, Ks, offs, offs4, finalize=True, dbg=False):
    import concourse.bass as bass
    import concourse.bacc as bacc
    import concourse.tile as tile
    from concourse import mybir
    from concourse.masks import make_identity

    N, F, H, ED, NC = cfg["N"], cfg["F"], cfg["H"], cfg["ED"], cfg["NC"]
    Np = N // NC
    n_tiles = len(Ks)
    FX = F + 2 * H  # gather-table row width (h | a_src | a_dst)
    S = int(offs[-1])
    S4 = S // 4
    f32 = mybir.dt.float32
    i32 = mybir.dt.int32
    AF = mybir.ActivationFunctionType
    OP = mybir.AluOpType
    EPS_IN, NEG = 1e-5, 0.2

    nc = bacc.Bacc("TRN2", target_bir_lowering=False, debug=False,
                   num_devices=NC)
    x_d = nc.declare_dram_parameter("x", [N, F], f32, isOutput=False)
    Wb_d = nc.declare_dram_parameter("Wb", [F, FX], f32, isOutput=False)
    v4_d = nc.declare_dram_parameter("v4", [4 * ED, 4 * H], f32, isOutput=False)
    idx_d = nc.declare_dram_parameter("idx", [S], i32, isOutput=False)
    ea4_d = nc.declare_dram_parameter("eaT4", [4 * ED, S4], f32, isOutput=False)
    rdeg_d = nc.declare_dram_parameter("rdeg", [n_tiles * P], f32, isOutput=False)
    idx0_d = nc.declare_dram_parameter("idx0", [n_tiles * P], i32, isOutput=False)
    gam_d = nc.declare_dram_parameter("gamma", [F], f32, isOutput=False)
    bet_d = nc.declare_dram_parameter("beta", [F], f32, isOutput=False)
    out_d = nc.declare_dram_parameter("out", [n_tiles * P, F], f32, isOutput=True)
    if dbg:
        dbg_op = nc.declare_dram_parameter("dbg_op", [P, n_tiles * F], f32,
                                           isOutput=True)
        dbg_acc = nc.declare_dram_parameter("dbg_acc", [P, 2], f32,
                                            isOutput=True)
        dbg_sg = nc.declare_dram_parameter("dbg_sg", [P, 2], f32,
                                           isOutput=True)

    with tile.TileContext(nc) as tc:
        with (
            tc.tile_pool(name="dram", bufs=1, space="DRAM") as dram,
            tc.tile_pool(name="consts", bufs=1) as consts,
            tc.tile_pool(name="ph_a", bufs=3) as pha,
            tc.tile_pool(name="ph_a_ps", bufs=2, space="PSUM") as pha_ps,
            tc.tile_pool(name="ph_b", bufs=2) as phb,
            tc.tile_pool(name="ph_b_ps", bufs=2, space="PSUM") as phb_ps,
            tc.tile_pool(name="stats_ps", bufs=1, space="PSUM") as stats_ps,
            tc.tile_pool(name="keep", bufs=1) as keep,
        ):
            hx = dram.tile([N + 1, FX], f32)

            ident = consts.tile([P, P], f32)
            make_identity(nc, ident[:])
            Wb_s = consts.tile([F, FX], f32)
            nc.sync.dma_start(out=Wb_s[:], in_=Wb_d[:, :])
            v4_s = consts.tile([4 * ED, 4 * H], f32)
            nc.sync.dma_start(out=v4_s[:], in_=v4_d[:, :])
            ones = consts.tile([P, 1], f32)
            nc.vector.memset(ones[:], 1.0)

            # ---------------- Phase A: hx = x @ Wb  (full table, each core)
            n_chunks = math.ceil(N / P)
            for i in range(n_chunks):
                r0 = i * P
                nrow = min(P, N - r0)
                x_t = pha.tile([P, F], f32, name="x_t")
                if nrow < P:
                    nc.vector.memset(x_t[:], 0.0)
                nc.sync.dma_start(out=x_t[:nrow, :], in_=x_d[r0:r0 + nrow, :])
                xT_p = pha_ps.tile([P, P], f32, name="xT_p")
                nc.tensor.transpose(out=xT_p[:], in_=x_t[:], identity=ident[:])
                xT_s = pha.tile([P, P], f32, name="xT_s")
                nc.vector.tensor_copy(out=xT_s[:], in_=xT_p[:])
                hx_p = pha_ps.tile([P, FX], f32, name="hx_p")
                nc.tensor.matmul(out=hx_p[:], lhsT=xT_s[:], rhs=Wb_s[:],
                                 start=True, stop=True)
                hx_s = pha.tile([P, FX], f32, name="hx_s")
                nc.vector.tensor_copy(out=hx_s[:], in_=hx_p[:])
                nc.sync.dma_start(out=hx[r0:r0 + nrow, :], in_=hx_s[:nrow, :])
            # dummy row N: zeros except a_src slots = -1e30
            dum = pha.tile([1, FX], f32, name="dum")
            nc.vector.memset(dum[:], 0.0)
            nc.vector.memset(dum[:, F:F + H], -1e30)
            nc.sync.dma_start(out=hx[N:N + 1, :], in_=dum[:])

            # ---------------- Phase B: per-tile attention + aggregation
            out_all = keep.tile([P, n_tiles, F], f32)
            acc = keep.tile([P, 2], f32)  # [channel, (sum, sumsq)]
            nc.vector.memset(acc[:], 0.0)

            for t in range(n_tiles):
                K = Ks[t]
                K4 = K // 4
                idx_t = phb.tile([P, K], i32, name="idx_t", tag="idx_t")
                nc.sync.dma_start(
                    out=idx_t[:],
                    in_=idx_d[int(offs[t]):int(offs[t + 1])].rearrange(
                        "(p k) -> p k", p=P),
                )
                g = phb.tile([P, K, FX], f32, name="g", tag="g")
                for k in range(K):
                    nc.gpsimd.indirect_dma_start(
                        out=g[:, k, :],
                        out_offset=None,
                        in_=hx[:, :],
                        in_offset=bass.IndirectOffsetOnAxis(
                            ap=idx_t[:, k:k + 1], axis=0),
                    )
                ea4_t = phb.tile([4 * ED, K4 * P], f32, name="ea4_t", tag="ea4_t")
                nc.sync.dma_start(
                    out=ea4_t[:],
                    in_=ea4_d[:, int(offs4[t]):int(offs4[t + 1])],
                )
                rdeg_t = phb.tile([P, 1], f32, name="rdeg_t", tag="rdeg_t")
                nc.sync.dma_start(out=rdeg_t[:],
                                  in_=rdeg_d[t * P:(t + 1) * P, None])

                # a_edge for all slots: quad matmuls [4ED,P] @ [4ED,4H]
                ae = phb.tile([P, K, H], f32, name="ae", tag="ae")
                QG = 16  # quads per PSUM bank (16*4H*4B = 2KB)
                for qg in range(math.ceil(K4 / QG)):
                    nq = min(QG, K4 - qg * QG)
                    ae_p = phb_ps.tile([P, QG * 4 * H], f32, name="ae_p",
                                       tag="ae_p")
                    for qi in range(nq):
                        q = qg * QG + qi
                        nc.tensor.matmul(
                            out=ae_p[:, qi * 4 * H:(qi + 1) * 4 * H],
                            lhsT=ea4_t[:, q * P:(q + 1) * P],
                            rhs=v4_s[:],
                            start=True, stop=True,
                        )
                    nc.vector.tensor_copy(
                        out=ae[:, qg * QG * 4:qg * QG * 4 + nq * 4, :],
                        in_=ae_p[:, :nq * 4 * H],
                    )
                # self-loop a_edge = (sum_k a_edge) * rdeg  (linearity)
                aeL = phb.tile([P, H], f32, name="aeL", tag="aeL")
                nc.vector.tensor_reduce(
                    out=aeL[:], in_=ae.transpose([0, 2, 1]),
                    axis=mybir.AxisListType.X, op=OP.add,
                )
                nc.vector.tensor_scalar_mul(ae[:, 0, :], aeL[:], rdeg_t[:])

                # alpha_pre = a_src[slots] + a_dst + a_edge   ([P, H, K] h-major)
                al = phb.tile([P, H, K], f32, name="al", tag="al")
                alv = al.transpose([0, 2, 1])  # iterate (k, h)
                nc.vector.tensor_tensor(
                    out=alv, in0=g[:, :, F:F + H], in1=ae[:, :, :], op=OP.add)
                adst = g[:, 0, F + H:F + 2 * H]  # [P, H] slot-0 row
                nc.vector.tensor_tensor(
                    out=alv, in0=alv,
                    in1=adst.unsqueeze(1).broadcast_to((P, K, H)), op=OP.add)
                # leaky_relu(z) = max(z, NEG*z)
                tl = phb.tile([P, H, K], f32, name="tl", tag="tl")
                nc.vector.tensor_scalar_mul(tl[:], al[:], NEG)
                nc.vector.tensor_tensor(out=al[:], in0=al[:], in1=tl[:],
                                        op=OP.max)
                # softmax over k per (p, h)
                amax = phb.tile([P, H], f32, name="amax", tag="amax")
                nc.vector.tensor_reduce(out=amax[:], in_=al[:],
                                        axis=mybir.AxisListType.X, op=OP.max)
                nc.vector.tensor_tensor(
                    out=al[:], in0=al[:],
                    in1=amax.unsqueeze(2).broadcast_to((P, H, K)),
                    op=OP.subtract)
                nc.scalar.activation(out=al[:], in_=al[:], func=AF.Exp)
                den = phb.tile([P, H], f32, name="den", tag="den")
                nc.vector.tensor_reduce(out=den[:], in_=al[:],
                                        axis=mybir.AxisListType.X, op=OP.add)
                rec = phb.tile([P, H], f32, name="rec", tag="rec")
                nc.vector.tensor_scalar_add(rec[:], den[:], 1e-16)
                nc.vector.reciprocal(rec[:], rec[:])
                nc.vector.tensor_tensor(
                    out=al[:], in0=al[:],
                    in1=rec.unsqueeze(2).broadcast_to((P, H, K)), op=OP.mult)

                # weighted message sum: g_h *= alpha ; out_pre = sum_k
                gh = g[:, :, 0:F].rearrange("p k (h d) -> p k h d", h=H)
                nc.vector.tensor_tensor(
                    out=gh, in0=gh,
                    in1=al.transpose([0, 2, 1]).unsqueeze(3)
                        .broadcast_to((P, K, H, F // H)),
                    op=OP.mult)
                op_t = out_all[:, t, :]  # [P, F]
                nc.vector.tensor_reduce(
                    out=op_t,
                    in_=g[:, :, 0:F].transpose([0, 2, 1]),  # [P, F, K]
                    axis=mybir.AxisListType.X, op=OP.add)

                # stats: per-channel sum & sumsq via ones-matmul (PE)
                sq = phb.tile([P, F], f32, name="sq", tag="sq")
                nc.vector.tensor_mul(sq[:], op_t, op_t)
                st_p = stats_ps.tile([P, 2], f32, name="st_p", tag="st_p",
                                     bufs=2)
                nc.tensor.matmul(out=st_p[:, 0:1], lhsT=op_t, rhs=ones[:],
                                 start=True, stop=True)
                nc.tensor.matmul(out=st_p[:, 1:2], lhsT=sq[:], rhs=ones[:],
                                 start=True, stop=True)
                nc.vector.tensor_add(acc[:], acc[:], st_p[:])

            # ---------------- Phase C: InstanceNorm stats allreduce + finalize
            st_in = dram.tile([P, 2], f32)
            st_out = dram.tile([P, 2], f32, addr_space="Shared")
            nc.sync.dma_start(out=st_in[:], in_=acc[:])
            nc.gpsimd.collective_compute(
                "AllReduce", OP.add,
                replica_groups=[list(range(NC))],
                ins=[st_in[:].opt()], outs=[st_out[:].opt()],
            )
            sg = keep.tile([P, 2], f32)
            nc.sync.dma_start(out=sg[:], in_=st_out[:])
            if dbg:
                nc.sync.dma_start(out=dbg_op[:, :],
                                  in_=out_all.rearrange("p t f -> p (t f)"))
                nc.sync.dma_start(out=dbg_acc[:, :], in_=acc[:])
                nc.sync.dma_start(out=dbg_sg[:, :], in_=sg[:])
            mean = keep.tile([P, 1], f32)
            nc.vector.tensor_scalar_mul(mean[:], sg[:, 0:1], 1.0 / N)
            ex2 = keep.tile([P, 1], f32)
            nc.vector.tensor_scalar_mul(ex2[:], sg[:, 1:2], 1.0 / N)
            var = keep.tile([P, 1], f32)
            nc.vector.tensor_mul(var[:], mean[:], mean[:])
            nc.vector.tensor_sub(var[:], ex2[:], var[:])
            rstd = keep.tile([P, 1], f32)
            eps_t = keep.tile([P, 1], f32)
            nc.vector.memset(eps_t[:], EPS_IN)
            nc.scalar.activation(out=rstd[:], in_=var[:], func=AF.Sqrt,
                                 bias=eps_t[:])
            nc.vector.reciprocal(rstd[:], rstd[:])
            gam_s = keep.tile([P, 1], f32)
            nc.sync.dma_start(out=gam_s[:], in_=gam_d[:, None])
            bet_s = keep.tile([P, 1], f32)
            nc.sync.dma_start(out=bet_s[:], in_=bet_d[:, None])
            scl = keep.tile([P, 1], f32)
            nc.vector.tensor_mul(scl[:], rstd[:], gam_s[:])
            bia = keep.tile([P, 1], f32)
            nc.vector.tensor_mul(bia[:], mean[:], scl[:])
            nc.vector.tensor_sub(bia[:], bet_s[:], bia[:])
            # broadcast scale/bias rows across partitions via DRAM round trip
            sb_dram = dram.tile([2, P], f32)
            nc.sync.dma_start(out=sb_dram[0, :], in_=scl[:, 0])
            nc.sync.dma_start(out=sb_dram[1, :], in_=bia[:, 0])
            sclB = keep.tile([P, F], f32)
            nc.sync.dma_start(out=sclB[:], in_=sb_dram[0:1, :].broadcast_to((P, P)))
            biaB = keep.tile([P, F], f32)
            nc.sync.dma_start(out=biaB[:], in_=sb_dram[1:2, :].broadcast_to((P, P)))

            with tc.tile_pool(name="ph_c", bufs=3) as phc:
                for t in range(n_tiles):
                    idx0_t = phc.tile([P, 1], i32, name="idx0_t")
                    nc.sync.dma_start(out=idx0_t[:],
                                      in_=idx0_d[t * P:(t + 1) * P, None])
                    xg = phc.tile([P, F], f32, name="xg")
                    nc.gpsimd.indirect_dma_start(
                        out=xg[:, :], out_offset=None,
                        in_=x_d[:, :],
                        in_offset=bass.IndirectOffsetOnAxis(ap=idx0_t[:, :],
                                                            axis=0),
                    )
                    z = phc.tile([P, F], f32, name="z")
                    nc.vector.tensor_mul(z[:], out_all[:, t, :], sclB[:])
                    nc.vector.tensor_add(z[:], z[:], biaB[:])
                    nc.vector.tensor_add(z[:], z[:], xg[:])
                    # elu(z) = max(z,0) + exp(min(z,0)) - 1
                    zm = phc.tile([P, F], f32, name="zm")
                    nc.vector.tensor_scalar_min(zm[:], z[:], 0.0)
                    nc.scalar.activation(out=zm[:], in_=zm[:], func=AF.Exp)
                    nc.vector.tensor_scalar_max(z[:], z[:], 0.0)
                    nc.vector.tensor_add(z[:], z[:], zm[:])
                    nc.vector.tensor_scalar_add(z[:], z[:], -1.0)
                    nc.sync.dma_start(out=out_d[t * P:(t + 1) * P, :], in_=z[:])
    if finalize:
        nc.finalize()
    return nc


# ---------------------------------------------------------------- driver
def _run_gat(x, edge_index, edge_attr, W, att_src, att_dst, W_e, att_edge,
             gamma, beta, cfg, trace=False, return_results=False):
    from concourse.bass_utils import run_bass_kernel_spmd

    N, F, H, Dh, NC = cfg["N"], cfg["F"], cfg["H"], cfg["Dh"], cfg["NC"]
    Np = N // NC
    Wb, v4 = _fold_weights(
        np.asarray(W, np.float32), np.asarray(att_src, np.float32),
        np.asarray(att_dst, np.float32), np.asarray(W_e, np.float32),
        np.asarray(att_edge, np.float32), H, Dh)
    cores, Ks, offs, offs4 = _preprocess(edge_index, edge_attr, cfg)
    nc = _build(cfg, Ks, offs, offs4)

    x_np = np.asarray(x, np.float32)
    gam = np.asarray(gamma, np.float32)
    bet = np.asarray(beta, np.float32)
    in_maps = []
    for c in range(NC):
        st = cores[c]["in"]
        in_maps.append(dict(x=x_np, Wb=Wb, v4=v4, idx=st["idx"],
                            eaT4=st["eaT4"], rdeg=st["rdeg"],
                            idx0=st["idx0"], gamma=gam, beta=bet))
    res = run_bass_kernel_spmd(nc, in_maps, core_ids=list(range(NC)),
                               trace=trace)
    out = np.empty((N, F), dtype=np.float32)
    for c in range(NC):
        oc = res.results[c]["out"]
        order = cores[c]["node_order"]
        real = order >= 0
        out[c * Np + order[real]] = oc[np.nonzero(real)[0]]
    if return_results:
        return out, res
    return out


def kernel(x, edge_index, edge_attr, W, att_src, att_dst, W_e, att_edge,
           gamma, beta):
    return _run_gat(x, edge_index, edge_attr, W, att_src, att_dst, W_e,
                    att_edge, gamma, beta, _cfg_full())
